# revision 19
# baseline (speedup 1.0000x reference)
"""XCA-style attention block (qkv 1x1 conv -> depthwise 3x3 -> L2-normed
cross-covariance attention -> 1x1 proj) on 8 TRN2 NeuronCores.

Sharding: core i handles (batch b = i//2, image half hf = i%2): 128 rows of
the 256-row image, plus one halo row for the depthwise conv. The L2 norms and
the per-head [24,24] Gram matrices are reductions over the full image, so each
pair of cores all-reduces a tiny [128,195] stats block; everything else is
local. Softmax + temperature + norm fixups are folded into a single [192,192]
matrix W2 = proj_w @ blockdiag(attn), so phase 2 is one matmul over v.
"""
import numpy as np
import ml_dtypes

import concourse.bass as bass
import concourse.tile as tile
from concourse import mybir
from concourse.bass_utils import run_bass_kernel_spmd
from concourse.masks import make_identity

# --- patch: this walrus build rejects >1 semaphore wait on a Drain ---------
import concourse.tile as _tile_mod
from concourse.vector_clock import ScopedClock as _SC, VectorClock as _VC


def _drain_and_barrier(self, tick_clock, wait_clock):
    gc = tick_clock.global_clock
    n = len(gc)
    nonzero = [i for i in range(n) if gc[i] > 0]
    for i in nonzero:
        vec = [gc[j] if j == i else 0 for j in range(n)]
        inst = self.nc.sync.drain()
        wait_clock.add_sem_waits(inst.ins, _SC({None: _VC(vec)}))
    if not nonzero:
        inst = self.nc.sync.drain()
        wait_clock.add_sem_waits(inst.ins, _SC({None: gc}))
    self.nc.all_engine_barrier()
    assert self.sems is not None
    popped = self.nc._tile_sem_poison_stack.pop()
    assert popped is self._sem_poison
    self.nc.clear_and_free_semaphores(list(self.sems.allocated().values()))
    self.nc.all_engine_barrier()


_tile_mod.TileContext._drain_and_barrier = _drain_and_barrier

# The same walrus limit applies to every engine instruction: at most ONE
# semaphore wait. Split extra waits onto preceding same-engine NoOps (engines
# execute in order, so earlier waits still gate the instruction). DMA copies
# use the descriptor path and tolerate multiple waits, so leave them alone.
_orig_commit_and_lower = _tile_mod.TileContext._commit_and_lower
_split_counter = [0]


def _commit_and_lower_split(self, inst, original_block, old_bb_map, bb_to_exit_bb):
    si = getattr(inst, "sync_info", None)
    if si is not None and len(si.on_wait) > 1 and inst.engine is not None:
        waits = list(si.on_wait)
        for w in waits[:-1]:
            _split_counter[0] += 1
            nop = mybir.InstNoOp(
                name=f"{inst.name}-wsplit{_split_counter[0]}",
                sync_info=mybir.SyncInfo(on_wait=[w], on_update=[]),
                bass_nofuse=True,
                engine=inst.engine,
            )
            self._commit_instruction(nop)
        inst.sync_info = mybir.SyncInfo(on_wait=[waits[-1]], on_update=list(si.on_update))
    return _orig_commit_and_lower(self, inst, original_block, old_bb_map, bb_to_exit_bb)


_tile_mod.TileContext._commit_and_lower = _commit_and_lower_split
# ---------------------------------------------------------------------------

F32 = mybir.dt.float32
BF16 = mybir.dt.bfloat16
AX = mybir.AxisListType
OP = mybir.AluOpType
ACTF = mybir.ActivationFunctionType

B, C, H, W = 4, 192, 256, 256
HEADS, HD = 8, 24
C3 = 3 * C  # 576
HALF = H // 2  # rows per core
CH = 8  # output rows per chunk
NCH = HALF // CH
RIN = CH + 2  # qkv rows computed per chunk (halo)
PX = CH * W
PXIN = RIN * W
CT = [128, 128, 128, 128, 64]  # qkv channel tiles
CTO = [0, 128, 256, 384, 512]
KT = [128, 64]  # contraction tiles over C=192
PAIRS = [[0, 1], [2, 3], [4, 5], [6, 7]]
NB = 512  # px per psum block

# depthwise tap -> engine ('v' = DVE stt, 't' = TensorE diag-matmul into PSUM).
# (0,1) must stay on 'v' (it is the accumulator-chain init).
TAP_ORDER = [(0, 1), (1, 1), (2, 1), (0, 0), (0, 2), (1, 0), (1, 2), (2, 0), (2, 2)]
TAP_ENG = {t: "v" for t in TAP_ORDER}


def build_nc():
    nc = bass.Bass()
    x_ext = nc.declare_dram_parameter("xin", [C, (HALF + 2) * W], BF16, isOutput=False)
    qkvwt_ext = nc.declare_dram_parameter("qkvwt", [C, C3], BF16, isOutput=False)
    projt_ext = nc.declare_dram_parameter("projt", [C, C], BF16, isOutput=False)
    dw9_ext = nc.declare_dram_parameter("dw9", [C3, 18], F32, isOutput=False)
    tempcol_ext = nc.declare_dram_parameter("tempcol", [128, 2], F32, isOutput=False)
    out_ext = nc.declare_dram_parameter("out", [C, HALF * W], BF16, isOutput=True)

    with tile.TileContext(nc) as tc:
        with tc.tile_pool(name="wpool", bufs=1) as wp, \
             tc.tile_pool(name="dram", bufs=1, space="DRAM") as dram:
            # ---- weights / constants
            qkvw0 = wp.tile([128, C3], BF16)
            qkvw1 = wp.tile([64, C3], BF16)
            nc.sync.dma_start(out=qkvw0[:], in_=qkvwt_ext[0:128, :])
            nc.sync.dma_start(out=qkvw1[:], in_=qkvwt_ext[128:192, :])
            projt0 = wp.tile([128, C], BF16)
            projt1 = wp.tile([64, C], BF16)
            nc.sync.dma_start(out=projt0[:], in_=projt_ext[0:128, :])
            nc.sync.dma_start(out=projt1[:], in_=projt_ext[128:192, :])
            dw9 = [wp.tile([CT[ct], 18], F32, name=f"dw9_{ct}") for ct in range(5)]
            for ct in range(5):
                nc.sync.dma_start(out=dw9[ct][:], in_=dw9_ext[CTO[ct]:CTO[ct] + CT[ct], :])
            tempcol = wp.tile([128, 2], F32)
            nc.sync.dma_start(out=tempcol[:], in_=tempcol_ext[:])
            ident = wp.tile([128, 128], BF16)
            make_identity(nc, ident[:])
            ident32 = wp.tile([128, 128], F32)
            make_identity(nc, ident32[:])
            ones32 = wp.tile([1, 128], F32)
            nc.vector.memset(ones32[:], 1.0)

            # persistent accumulators
            sq_part = [wp.tile([CT[ct], NCH], F32, name=f"sqp{ct}") for ct in range(3)]
            v_dram = dram.tile([C, HALF * W], BF16)
            stats = wp.tile([128, 195], F32)

            # ================= phase 1 =================
            with tc.tile_pool(name="p1", bufs=2) as p1, \
                 tc.tile_pool(name="ps1", bufs=2, space="PSUM") as ps1, \
                 tc.tile_pool(name="gps", bufs=1, space="PSUM") as gps:
                gA = gps.tile([96, 96], F32, tag="gA")
                gB = gps.tile([96, 96], F32, tag="gB")

                for c in range(NCH):
                    xrow0 = c * CH * W
                    x0 = p1.tile([128, PXIN], BF16, tag="x0", bufs=3)
                    x1 = p1.tile([64, PXIN], BF16, tag="x1", bufs=3)
                    nc.sync.dma_start(out=x0[:], in_=x_ext[0:128, xrow0:xrow0 + PXIN])
                    nc.sync.dma_start(out=x1[:], in_=x_ext[128:192, xrow0:xrow0 + PXIN])

                    # qkv 1x1 conv for RIN rows (chunk + halo)
                    qkv = [p1.tile([CT[ct], PXIN], BF16, tag=f"qkv{ct}", name=f"qkv{ct}") for ct in range(5)]
                    for ct in range(5):
                        w0 = qkvw0[:, CTO[ct]:CTO[ct] + CT[ct]]
                        w1 = qkvw1[:, CTO[ct]:CTO[ct] + CT[ct]]
                        for nb in range(PXIN // NB):
                            ps = ps1.tile([128, NB], F32, tag="qkvps")
                            o = ps[0:CT[ct], :]
                            nc.tensor.matmul(o, w0, x0[:, nb * NB:(nb + 1) * NB], start=True, stop=False)
                            nc.tensor.matmul(o, w1, x1[:, nb * NB:(nb + 1) * NB], start=False, stop=True)
                            nc.scalar.copy(qkv[ct][:, nb * NB:(nb + 1) * NB], o)

                    # Depthwise 3x3 as 9 full-width per-channel FMAs. Side taps
                    # (dx != 1) read a 1-element-shifted copy (qkvB, data at
                    # offset +1) so every operand is 4B-aligned and the DVE
                    # runs in its 2x bf16 mode. Full-width side taps wrap at
                    # row boundaries; tiny strided "patch" ops subtract the
                    # wrapped contribution (cols 0 / 255), matching SAME pad.
                    acc = [p1.tile([CT[ct], PX], BF16, tag=f"acc{ct}", name=f"acc{ct}") for ct in range(5)]
                    for ct in range(5):
                        qB = p1.tile([CT[ct], PXIN + 2], BF16, tag=f"qkvB{ct}", name=f"qkvB{ct}")
                        nc.gpsimd.memset(qB[:, 0:1], 0.0)
                        nc.gpsimd.memset(qB[:, PXIN + 1:PXIN + 2], 0.0)
                        nc.sync.dma_start(out=qB[:, 1:PXIN + 1], in_=qkv[ct][:])
                        for ti, (dy, dx) in enumerate(TAP_ORDER):
                            if TAP_ENG[(dy, dx)] != "v":
                                continue
                            wcol = dw9[ct][:, 3 * dy + dx:3 * dy + dx + 1]
                            if dx == 1:
                                src = qkv[ct][:, dy * W:dy * W + PX]
                            else:
                                src = qB[:, dy * W + dx:dy * W + dx + PX]
                            dst = acc[ct][:]
                            if ti == 0:
                                nc.vector.tensor_scalar(
                                    out=dst, in0=src, scalar1=wcol, scalar2=None,
                                    op0=OP.mult)
                            else:
                                nc.vector.scalar_tensor_tensor(
                                    out=dst, in0=src, scalar=wcol, in1=dst,
                                    op0=OP.mult, op1=OP.add)
                        # wrap-correction patches for the side taps: the
                        # full-width tap dx=0 added w*A[(r+dy)*W - 1] at col 0
                        # (prev row's col 255) and dx=2 added w*A[(r+dy+1)*W]
                        # at col 255 (next row's col 0); subtract those.
                        a3 = acc[ct][:].rearrange("p (r w) -> p r w", w=W)

                        def col_view(a, _qB=qB):
                            # [P, CH, 1] view of elements _qB[:, a + r*W]
                            s = a if a + CH * W <= PXIN + 2 else PXIN + 2 - CH * W
                            d = a - s
                            v = _qB[:, s:s + CH * W].rearrange("p (r w) -> p r w", w=W)
                            return v[:, :, d:d + 1]

                        for dy in range(3):
                            if TAP_ENG[(dy, 0)] == "v":
                                nwcol = dw9[ct][:, 9 + 3 * dy:10 + 3 * dy]
                                nc.vector.scalar_tensor_tensor(
                                    out=a3[:, :, 0:1], in0=col_view(dy * W),
                                    scalar=nwcol, in1=a3[:, :, 0:1],
                                    op0=OP.mult, op1=OP.add)
                            if TAP_ENG[(dy, 2)] == "v":
                                nwcol = dw9[ct][:, 11 + 3 * dy:12 + 3 * dy]
                                nc.vector.scalar_tensor_tensor(
                                    out=a3[:, :, W - 1:W], in0=col_view(1 + (dy + 1) * W),
                                    scalar=nwcol, in1=a3[:, :, W - 1:W],
                                    op0=OP.mult, op1=OP.add)

                    # sum of squares for q,k channel tiles (ct 0,1,2)
                    for ct in range(3):
                        scr = p1.tile([CT[ct], PX], BF16, tag="sqscr")
                        nc.scalar.activation(
                            out=scr[:], in_=acc[ct][:], func=ACTF.Square,
                            accum_out=sq_part[ct][:, c:c + 1])

                    # spill v
                    nc.sync.dma_start(out=v_dram[0:128, c * PX:(c + 1) * PX], in_=acc[3][:])
                    nc.sync.dma_start(out=v_dram[128:192, c * PX:(c + 1) * PX], in_=acc[4][:])

                    # transposes + gram accumulation per 128-px block.
                    # Layout qkt = [q ch 0:192 | k ch 0:192] so the three
                    # 128-wide transpose blocks land contiguously and one
                    # wide copy moves PSUM->SBUF.
                    for pb in range(PX // 128):
                        qkt = p1.tile([128, 2 * C], BF16, tag="qkt", bufs=3)
                        pbs = slice(pb * 128, (pb + 1) * 128)
                        tg = ps1.tile([128, 384], BF16, tag="tps", bufs=2)
                        nc.tensor.transpose(tg[:, 0:128], acc[0][:, pbs], ident[:])
                        nc.tensor.transpose(tg[:, 128:256], acc[1][:, pbs], ident[:])
                        nc.tensor.transpose(tg[:, 256:384], acc[2][:, pbs], ident[:])
                        nc.scalar.copy(qkt[:], tg[:])

                        first = (c == 0 and pb == 0)
                        last = (c == NCH - 1 and pb == PX // 128 - 1)
                        nc.tensor.matmul(gA[:], qkt[:, 0:96], qkt[:, 192:288],
                                         start=first, stop=last)
                        nc.tensor.matmul(gB[:], qkt[:, 96:192], qkt[:, 288:384],
                                         start=first, stop=last)

                # fold chunk partials; pack stats = [gA | gB | ssq]
                for ct in range(3):
                    nc.vector.tensor_reduce(
                        out=stats[0:CT[ct], 192 + ct:193 + ct],
                        in_=sq_part[ct][:], axis=AX.X, op=OP.add)
                nc.scalar.copy(stats[0:96, 0:96], gA[:])
                nc.scalar.copy(stats[0:96, 96:192], gB[:])

            # ================= collective =================
            cc_in = dram.tile([128, 195], F32)
            cc_out = dram.tile([128, 195], F32)
            nc.sync.dma_start(out=cc_in[:], in_=stats[:])
            nc.gpsimd.collective_compute(
                "AllReduce", OP.add, replica_groups=PAIRS,
                ins=[cc_in.opt()], outs=[cc_out.opt()])
            statf = wp.tile([128, 195], F32)
            nc.sync.dma_start(out=statf[:], in_=cc_out[:])

            # ================= epilogue (tiny) =================
            with tc.tile_pool(name="ep", bufs=1) as ep, \
                 tc.tile_pool(name="eps", bufs=1, space="PSUM") as eps:
                # 1/max(sqrt(ssq), eps) per q/k channel, [128, 3] by ct
                nrm = ep.tile([128, 3], F32)
                nc.scalar.activation(out=nrm[:], in_=statf[:, 192:195], func=ACTF.Sqrt)
                nc.vector.tensor_scalar(out=nrm[:], in0=nrm[:], scalar1=1e-12,
                                        scalar2=None, op0=OP.max)
                rn = ep.tile([128, 3], F32)
                nc.vector.reciprocal(rn[:], nrm[:])

                # row scales (q-norms * temperature), partition-packed per gram tile
                rsA = ep.tile([96, 1], F32)
                nc.vector.tensor_tensor(out=rsA[:], in0=rn[0:96, 0:1],
                                        in1=tempcol[0:96, 0:1], op=OP.mult)
                # partition-offset rearrangements go through SBUF->SBUF DMA:
                # DVE writes at non-quadrant-aligned partition bases are illegal
                rsB = ep.tile([96, 1], F32)
                nc.sync.dma_start(out=rsB[0:32, :], in_=rn[96:128, 0:1])
                nc.sync.dma_start(out=rsB[32:96, :], in_=rn[0:64, 1:2])
                nc.vector.tensor_tensor(out=rsB[:], in0=rsB[:],
                                        in1=tempcol[0:96, 1:2], op=OP.mult)

                # column scales (k-norms) -> broadcast [96, 96] via rank-1 matmul
                rkc = ep.tile([96, 2], F32)
                nc.sync.dma_start(out=rkc[0:64, 0:1], in_=rn[64:128, 1:2])
                nc.sync.dma_start(out=rkc[64:96, 0:1], in_=rn[0:32, 2:3])
                nc.sync.dma_start(out=rkc[0:96, 1:2], in_=rn[32:128, 2:3])
                tps = eps.tile([2, 96], F32, tag="t")
                nc.tensor.transpose(tps[:], rkc[:], ident32[0:96, 0:96])
                rkrs = ep.tile([2, 96], F32)
                nc.vector.tensor_copy(rkrs[:], tps[:])
                rkr = [ep.tile([1, 96], F32, name=f"rkr{g}") for g in range(2)]
                nc.vector.tensor_copy(rkr[0][:], rkrs[0:1, :])
                nc.sync.dma_start(out=rkr[1][:], in_=rkrs[1:2, :])
                bcps = eps.tile([96, 96], F32, tag="bc")
                bc = [ep.tile([96, 96], F32, name=f"bc{g}") for g in range(2)]
                for g in range(2):
                    nc.tensor.matmul(bcps[:], ones32[0:1, 0:96], rkr[g][:],
                                     start=True, stop=True)
                    nc.vector.tensor_copy(bc[g][:], bcps[:])

                # logits = gram * rq * rk * temp; diag-extract -> [96, 24] per tile
                attn = []
                for g in range(2):
                    lg = ep.tile([96, 96], F32, name=f"lg{g}")
                    nc.vector.tensor_scalar(out=lg[:], in0=statf[0:96, 96 * g:96 * (g + 1)],
                                            scalar1=(rsA if g == 0 else rsB)[:],
                                            scalar2=None, op0=OP.mult)
                    nc.vector.tensor_tensor(out=lg[:], in0=lg[:], in1=bc[g][:], op=OP.mult)
                    sm = ep.tile([96, HD], F32, name=f"sm{g}")
                    for hl in range(4):
                        nc.sync.dma_start(out=sm[24 * hl:24 * (hl + 1), :],
                                          in_=lg[24 * hl:24 * (hl + 1), 24 * hl:24 * (hl + 1)])
                    mx = ep.tile([96, 1], F32, name=f"mx{g}")
                    nc.vector.tensor_reduce(out=mx[:], in_=sm[:], axis=AX.X, op=OP.max)
                    nc.vector.tensor_scalar(out=sm[:], in0=sm[:], scalar1=mx[:],
                                            scalar2=None, op0=OP.subtract)
                    ex = ep.tile([96, HD], F32, name=f"ex{g}")
                    nc.scalar.activation(out=ex[:], in_=sm[:], func=ACTF.Exp)
                    sme = ep.tile([96, 1], F32, name=f"sme{g}")
                    nc.vector.tensor_reduce(out=sme[:], in_=ex[:], axis=AX.X, op=OP.add)
                    rs = ep.tile([96, 1], F32, name=f"rs{g}")
                    nc.vector.reciprocal(rs[:], sme[:])
                    at = ep.tile([96, HD], BF16, name=f"at{g}")
                    nc.vector.tensor_scalar(out=at[:], in0=ex[:], scalar1=rs[:],
                                            scalar2=None, op0=OP.mult)
                    attn.append(at)

                # blockdiag(attn) as lhsT rows=out-chan(24h+d), cols=v-chan(24h+e)
                abd0 = ep.tile([128, C], BF16)
                abd1 = ep.tile([64, C], BF16)
                nc.vector.memset(abd0[:], 0.0)
                nc.vector.memset(abd1[:], 0.0)
                for h in range(HEADS):
                    g, hl = divmod(h, 4)
                    src = attn[g]
                    r0, cc0 = 24 * h, 24 * h
                    if r0 + 24 <= 128:
                        nc.sync.dma_start(out=abd0[r0:r0 + 24, cc0:cc0 + 24],
                                          in_=src[24 * hl:24 * hl + 24, :])
                    elif r0 >= 128:
                        nc.sync.dma_start(out=abd1[r0 - 128:r0 - 104, cc0:cc0 + 24],
                                          in_=src[24 * hl:24 * hl + 24, :])
                    else:
                        k0 = 128 - r0
                        nc.sync.dma_start(out=abd0[r0:128, cc0:cc0 + 24],
                                          in_=src[24 * hl:24 * hl + k0, :])
                        nc.sync.dma_start(out=abd1[0:24 - k0, cc0:cc0 + 24],
                                          in_=src[24 * hl + k0:24 * hl + 24, :])

                # W2T[c, o] = sum_r abd[r, c] * projt[r, o]
                w2t0 = ep.tile([128, C], BF16)
                w2t1 = ep.tile([64, C], BF16)
                wps = eps.tile([128, C], F32, tag="wps")
                nc.tensor.matmul(wps[:], abd0[:, 0:128], projt0[:], start=True, stop=False)
                nc.tensor.matmul(wps[:], abd1[:, 0:128], projt1[:], start=False, stop=True)
                nc.scalar.copy(w2t0[:], wps[:])
                wps2 = eps.tile([64, C], F32, tag="wps2")
                nc.tensor.matmul(wps2[:], abd0[:, 128:192], projt0[:], start=True, stop=False)
                nc.tensor.matmul(wps2[:], abd1[:, 128:192], projt1[:], start=False, stop=True)
                nc.scalar.copy(w2t1[:], wps2[:])

                # ================= phase 2: out = W2 @ v =================
                with tc.tile_pool(name="p2", bufs=2) as p2, \
                     tc.tile_pool(name="ps2", bufs=2, space="PSUM") as ps2:
                    for c in range(NCH):
                        cs = slice(c * PX, (c + 1) * PX)
                        vb0 = p2.tile([128, PX], BF16, tag="vb0")
                        vb1 = p2.tile([64, PX], BF16, tag="vb1")
                        nc.sync.dma_start(out=vb0[:], in_=v_dram[0:128, cs])
                        nc.sync.dma_start(out=vb1[:], in_=v_dram[128:192, cs])
                        ob0 = p2.tile([128, PX], BF16, tag="ob0")
                        ob1 = p2.tile([64, PX], BF16, tag="ob1")
                        for nb in range(PX // NB):
                            nbs = slice(nb * NB, (nb + 1) * NB)
                            f0 = ps2.tile([128, NB], F32, tag="f0")
                            nc.tensor.matmul(f0[:], w2t0[:, 0:128], vb0[:, nbs], start=True, stop=False)
                            nc.tensor.matmul(f0[:], w2t1[:, 0:128], vb1[:, nbs], start=False, stop=True)
                            nc.scalar.copy(ob0[:, nbs], f0[:])
                            f1 = ps2.tile([64, NB], F32, tag="f1")
                            nc.tensor.matmul(f1[:], w2t0[:, 128:192], vb0[:, nbs], start=True, stop=False)
                            nc.tensor.matmul(f1[:], w2t1[:, 128:192], vb1[:, nbs], start=False, stop=True)
                            nc.scalar.copy(ob1[:, nbs], f1[:])
                        nc.sync.dma_start(out=out_ext[0:128, cs], in_=ob0[:])
                        nc.sync.dma_start(out=out_ext[128:192, cs], in_=ob1[:])
    return nc


_NC_CACHE = None


def _get_nc():
    global _NC_CACHE
    if _NC_CACHE is None:
        _NC_CACHE = build_nc()
    return _NC_CACHE


def _shard_inputs(x, qkv_w, dw_w, proj_w, temperature):
    qkvwt = np.ascontiguousarray(qkv_w.T).astype(ml_dtypes.bfloat16)
    projt = np.ascontiguousarray(proj_w.T).astype(ml_dtypes.bfloat16)
    dw9_ = dw_w.reshape(C3, 9).astype(np.float32)
    dw9 = np.ascontiguousarray(np.concatenate([dw9_, -dw9_], axis=1))
    temp = np.asarray(temperature).reshape(HEADS)
    tempcol = np.zeros((128, 2), np.float32)
    for h in range(HEADS):
        g, hl = divmod(h, 4)
        tempcol[24 * hl:24 * (hl + 1), g] = temp[h]

    in_maps = []
    for i in range(8):
        b, hf = divmod(i, 2)
        xin = np.zeros((C, HALF + 2, W), np.float32)
        r0 = hf * HALF - 1
        lo, hi = max(r0, 0), min(r0 + HALF + 2, H)
        xin[:, lo - r0:hi - r0, :] = x[b, :, lo:hi, :]
        in_maps.append({
            "xin": xin.reshape(C, (HALF + 2) * W).astype(ml_dtypes.bfloat16),
            "qkvwt": qkvwt, "projt": projt, "dw9": dw9, "tempcol": tempcol,
        })
    return in_maps


def kernel(x, qkv_w, dw_w, proj_w, temperature):
    nc = _get_nc()
    in_maps = _shard_inputs(x, qkv_w, dw_w, proj_w, temperature)
    res = run_bass_kernel_spmd(nc, in_maps, core_ids=list(range(8)))
    out = np.empty((B, C, H, W), np.float32)
    for i in range(8):
        b, hf = divmod(i, 2)
        o = res.results[i]["out"].astype(np.float32).reshape(C, HALF, W)
        out[b, :, hf * HALF:(hf + 1) * HALF, :] = o
    return out


# revision 25
# speedup vs baseline: 1.7542x; 1.7542x over previous
"""XCA-style attention block (qkv 1x1 conv -> depthwise 3x3 -> L2-normed
cross-covariance attention -> 1x1 proj) on 8 TRN2 NeuronCores.

Sharding: core i handles (batch b = i//2, image half hf = i%2): 128 rows of
the 256-row image, plus one halo row for the depthwise conv. The L2 norms and
the per-head [24,24] Gram matrices are reductions over the full image, so each
pair of cores all-reduces a tiny [128,195] stats block; everything else is
local. Softmax + temperature + norm fixups are folded into a single [192,192]
matrix W2 = proj_w @ blockdiag(attn), so phase 2 is one matmul over v.
"""
import numpy as np
import ml_dtypes

import concourse.bass as bass
import concourse.tile as tile
from concourse import mybir
from concourse.bass_utils import run_bass_kernel_spmd
from concourse.masks import make_identity

# --- patch: this walrus build rejects >1 semaphore wait on a Drain ---------
import concourse.tile as _tile_mod
from concourse.vector_clock import ScopedClock as _SC, VectorClock as _VC


def _drain_and_barrier(self, tick_clock, wait_clock):
    gc = tick_clock.global_clock
    n = len(gc)
    nonzero = [i for i in range(n) if gc[i] > 0]
    for i in nonzero:
        vec = [gc[j] if j == i else 0 for j in range(n)]
        inst = self.nc.sync.drain()
        wait_clock.add_sem_waits(inst.ins, _SC({None: _VC(vec)}))
    if not nonzero:
        inst = self.nc.sync.drain()
        wait_clock.add_sem_waits(inst.ins, _SC({None: gc}))
    self.nc.all_engine_barrier()
    assert self.sems is not None
    popped = self.nc._tile_sem_poison_stack.pop()
    assert popped is self._sem_poison
    self.nc.clear_and_free_semaphores(list(self.sems.allocated().values()))
    self.nc.all_engine_barrier()


_tile_mod.TileContext._drain_and_barrier = _drain_and_barrier

# The same walrus limit applies to every engine instruction: at most ONE
# semaphore wait. Split extra waits onto preceding same-engine NoOps (engines
# execute in order, so earlier waits still gate the instruction). DMA copies
# use the descriptor path and tolerate multiple waits, so leave them alone.
_orig_commit_and_lower = _tile_mod.TileContext._commit_and_lower
_split_counter = [0]


def _commit_and_lower_split(self, inst, original_block, old_bb_map, bb_to_exit_bb):
    si = getattr(inst, "sync_info", None)
    if si is not None and len(si.on_wait) > 1 and inst.engine is not None:
        waits = list(si.on_wait)
        for w in waits[:-1]:
            _split_counter[0] += 1
            nop = mybir.InstNoOp(
                name=f"{inst.name}-wsplit{_split_counter[0]}",
                sync_info=mybir.SyncInfo(on_wait=[w], on_update=[]),
                bass_nofuse=True,
                engine=inst.engine,
            )
            self._commit_instruction(nop)
        inst.sync_info = mybir.SyncInfo(on_wait=[waits[-1]], on_update=list(si.on_update))
    return _orig_commit_and_lower(self, inst, original_block, old_bb_map, bb_to_exit_bb)


_tile_mod.TileContext._commit_and_lower = _commit_and_lower_split
# ---------------------------------------------------------------------------

F32 = mybir.dt.float32
BF16 = mybir.dt.bfloat16
AX = mybir.AxisListType
OP = mybir.AluOpType
ACTF = mybir.ActivationFunctionType

B, C, H, W = 4, 192, 256, 256
HEADS, HD = 8, 24
C3 = 3 * C  # 576
HALF = H // 2  # rows per core
CH = 8  # output rows per chunk
NCH = HALF // CH
RIN = CH + 2  # qkv rows computed per chunk (halo)
PX = CH * W
PXIN = RIN * W
CT = [128, 128, 128, 128, 64]  # qkv channel tiles
CTO = [0, 128, 256, 384, 512]
KT = [128, 64]  # contraction tiles over C=192
PAIRS = [[0, 1], [2, 3], [4, 5], [6, 7]]
NB = 512  # px per psum block

# depthwise tap split: 5 side taps on TensorE (diag matmuls, PSUM-accumulated),
# center (0,1) fused with the PSUM merge on DVE, rest as DVE mult+add pairs.
PE_TAPS = [(0, 0), (0, 2), (1, 0), (1, 2), (2, 0)]
DVE_PAIR_TAPS = [(1, 1), (2, 1), (2, 2)]


def build_nc():
    nc = bass.Bass()
    x_ext = nc.declare_dram_parameter("xin", [C, (HALF + 2) * W], BF16, isOutput=False)
    qkvwt_ext = nc.declare_dram_parameter("qkvwt", [C, C3], BF16, isOutput=False)
    projt_ext = nc.declare_dram_parameter("projt", [C, C], BF16, isOutput=False)
    dw9_ext = nc.declare_dram_parameter("dw9", [C3, 18], F32, isOutput=False)
    dwd_ext = nc.declare_dram_parameter("dwdiag", [len(PE_TAPS) * 128, C3], BF16, isOutput=False)
    tempcol_ext = nc.declare_dram_parameter("tempcol", [128, 2], F32, isOutput=False)
    out_ext = nc.declare_dram_parameter("out", [C, HALF * W], BF16, isOutput=True)

    with tile.TileContext(nc) as tc:
        with tc.tile_pool(name="wpool", bufs=1) as wp, \
             tc.tile_pool(name="dram", bufs=1, space="DRAM") as dram:
            # ---- weights / constants
            qkvw0 = wp.tile([128, C3], BF16)
            qkvw1 = wp.tile([64, C3], BF16)
            nc.sync.dma_start(out=qkvw0[:], in_=qkvwt_ext[0:128, :])
            nc.sync.dma_start(out=qkvw1[:], in_=qkvwt_ext[128:192, :])
            projt0 = wp.tile([128, C], BF16)
            projt1 = wp.tile([64, C], BF16)
            nc.sync.dma_start(out=projt0[:], in_=projt_ext[0:128, :])
            nc.sync.dma_start(out=projt1[:], in_=projt_ext[128:192, :])
            dw9 = [wp.tile([CT[ct], 18], F32, name=f"dw9_{ct}") for ct in range(5)]
            for ct in range(5):
                nc.sync.dma_start(out=dw9[ct][:], in_=dw9_ext[CTO[ct]:CTO[ct] + CT[ct], :])
            tempcol = wp.tile([128, 2], F32)
            nc.sync.dma_start(out=tempcol[:], in_=tempcol_ext[:])
            dwd = [wp.tile([128, C3], BF16, name=f"dwd{s}") for s in range(len(PE_TAPS))]
            for s in range(len(PE_TAPS)):
                nc.sync.dma_start(out=dwd[s][:], in_=dwd_ext[128 * s:128 * (s + 1), :])
            ident = wp.tile([128, 128], BF16)
            make_identity(nc, ident[:])
            ident32 = wp.tile([128, 128], F32)
            make_identity(nc, ident32[:])
            ones32 = wp.tile([1, 128], F32)
            nc.vector.memset(ones32[:], 1.0)

            # persistent accumulators
            sq_part = [wp.tile([CT[ct], NCH], F32, name=f"sqp{ct}") for ct in range(3)]
            v_dram = dram.tile([C, HALF * W], BF16)
            stats = wp.tile([128, 195], F32)

            # ================= phase 1 =================
            with tc.tile_pool(name="p1", bufs=2) as p1, \
                 tc.tile_pool(name="ps1", bufs=2, space="PSUM") as ps1, \
                 tc.tile_pool(name="gps", bufs=1, space="PSUM") as gps:
                gA = gps.tile([96, 96], F32, tag="gA")
                gB = gps.tile([96, 96], F32, tag="gB")

                for c in range(NCH):
                    xrow0 = c * CH * W
                    x0 = p1.tile([128, PXIN], BF16, tag="x0", bufs=3)
                    x1 = p1.tile([64, PXIN], BF16, tag="x1", bufs=3)
                    nc.sync.dma_start(out=x0[:], in_=x_ext[0:128, xrow0:xrow0 + PXIN])
                    nc.sync.dma_start(out=x1[:], in_=x_ext[128:192, xrow0:xrow0 + PXIN])

                    # qkv 1x1 conv for RIN rows (chunk + halo)
                    qkv = [p1.tile([CT[ct], PXIN], BF16, tag=f"qkv{ct}", name=f"qkv{ct}") for ct in range(5)]
                    for ct in range(5):
                        w0 = qkvw0[:, CTO[ct]:CTO[ct] + CT[ct]]
                        w1 = qkvw1[:, CTO[ct]:CTO[ct] + CT[ct]]
                        for nb in range(PXIN // NB):
                            ps = ps1.tile([128, NB], F32, tag="qkvps")
                            o = ps[0:CT[ct], :]
                            nc.tensor.matmul(o, w0, x0[:, nb * NB:(nb + 1) * NB], start=True, stop=False)
                            nc.tensor.matmul(o, w1, x1[:, nb * NB:(nb + 1) * NB], start=False, stop=True)
                            nc.scalar.copy(qkv[ct][:, nb * NB:(nb + 1) * NB], o)

                    # Depthwise 3x3 as 9 full-width per-channel taps.
                    # - PE_TAPS run as diagonal-weight matmuls accumulating in
                    #   PSUM (dwp); the DVE "merge" tap folds that PSUM block
                    #   plus the (0,1) center tap into the SBUF accumulator.
                    # - DVE pair taps: tensor_scalar mult (fast mode) into a
                    #   temp + tensor_tensor add (2x mode).
                    # Side taps read qkvB (data at +1 elem) so operands stay
                    # 4B-aligned; full-width side taps wrap at row boundaries
                    # and tiny strided patch ops subtract the wrapped part.
                    acc = [p1.tile([CT[ct], PX], BF16, tag=f"acc{ct}", name=f"acc{ct}") for ct in range(5)]
                    for ct in range(5):
                        qB = p1.tile([CT[ct], PXIN + 2], BF16, tag=f"qkvB{ct}", name=f"qkvB{ct}")
                        nc.gpsimd.memset(qB[:, 0:1], 0.0)
                        nc.gpsimd.memset(qB[:, PXIN + 1:PXIN + 2], 0.0)
                        nc.sync.dma_start(out=qB[:, 1:PXIN + 1], in_=qkv[ct][:])

                        w01 = dw9[ct][:, 1:2]
                        for nb in range(PX // NB):
                            dwp = ps1.tile([128, NB], F32, tag="dwps")
                            o = dwp[0:CT[ct], :]
                            for si, (dy, dx) in enumerate(PE_TAPS):
                                src = qB[:, dy * W + dx + nb * NB:dy * W + dx + (nb + 1) * NB]
                                nc.tensor.matmul(o, dwd[si][0:CT[ct], CTO[ct]:CTO[ct] + CT[ct]],
                                                 src, start=(si == 0),
                                                 stop=(si == len(PE_TAPS) - 1))
                            # merge + center tap (0,1): acc = qkv*w + dwp
                            nc.vector.scalar_tensor_tensor(
                                out=acc[ct][:, nb * NB:(nb + 1) * NB],
                                in0=qkv[ct][:, nb * NB:(nb + 1) * NB],
                                scalar=w01, in1=o, op0=OP.mult, op1=OP.add)

                        for (dy, dx) in DVE_PAIR_TAPS:
                            wcol = dw9[ct][:, 3 * dy + dx:3 * dy + dx + 1]
                            if dx == 1:
                                src = qkv[ct][:, dy * W:dy * W + PX]
                            else:
                                src = qB[:, dy * W + dx:dy * W + dx + PX]
                            tmp = p1.tile([CT[ct], PX], BF16, tag="dwtmp", name="dwtmp")
                            nc.vector.tensor_scalar(out=tmp[:], in0=src, scalar1=wcol,
                                                    scalar2=None, op0=OP.mult)
                            nc.vector.tensor_tensor(out=acc[ct][:], in0=acc[ct][:],
                                                    in1=tmp[:], op=OP.add)

                        # wrap-correction patches for all six side taps: the
                        # full-width tap dx=0 added w*A[(r+dy)*W - 1] at col 0
                        # (prev row's col 255) and dx=2 added w*A[(r+dy+1)*W]
                        # at col 255 (next row's col 0); subtract those.
                        a3 = acc[ct][:].rearrange("p (r w) -> p r w", w=W)

                        def col_view(a, _qB=qB):
                            # [P, CH, 1] view of elements _qB[:, a + r*W]
                            s = a if a + CH * W <= PXIN + 2 else PXIN + 2 - CH * W
                            d = a - s
                            v = _qB[:, s:s + CH * W].rearrange("p (r w) -> p r w", w=W)
                            return v[:, :, d:d + 1]

                        for dy in range(3):
                            nwcol = dw9[ct][:, 9 + 3 * dy:10 + 3 * dy]
                            nc.vector.scalar_tensor_tensor(
                                out=a3[:, :, 0:1], in0=col_view(dy * W),
                                scalar=nwcol, in1=a3[:, :, 0:1],
                                op0=OP.mult, op1=OP.add)
                            nwcol = dw9[ct][:, 11 + 3 * dy:12 + 3 * dy]
                            nc.vector.scalar_tensor_tensor(
                                out=a3[:, :, W - 1:W], in0=col_view(1 + (dy + 1) * W),
                                scalar=nwcol, in1=a3[:, :, W - 1:W],
                                op0=OP.mult, op1=OP.add)

                    # sum of squares for q,k channel tiles (ct 0,1,2)
                    for ct in range(3):
                        scr = p1.tile([CT[ct], PX], BF16, tag="sqscr")
                        nc.scalar.activation(
                            out=scr[:], in_=acc[ct][:], func=ACTF.Square,
                            accum_out=sq_part[ct][:, c:c + 1])

                    # spill v
                    nc.sync.dma_start(out=v_dram[0:128, c * PX:(c + 1) * PX], in_=acc[3][:])
                    nc.sync.dma_start(out=v_dram[128:192, c * PX:(c + 1) * PX], in_=acc[4][:])

                    # transposes + gram accumulation per 128-px block.
                    # Layout qkt = [q ch 0:192 | k ch 0:192] so the three
                    # 128-wide transpose blocks land contiguously and one
                    # wide copy moves PSUM->SBUF.
                    for pb in range(PX // 128):
                        qkt = p1.tile([128, 2 * C], BF16, tag="qkt", bufs=3)
                        pbs = slice(pb * 128, (pb + 1) * 128)
                        tg = ps1.tile([128, 384], BF16, tag="tps", bufs=2)
                        nc.tensor.transpose(tg[:, 0:128], acc[0][:, pbs], ident[:])
                        nc.tensor.transpose(tg[:, 128:256], acc[1][:, pbs], ident[:])
                        nc.tensor.transpose(tg[:, 256:384], acc[2][:, pbs], ident[:])
                        nc.scalar.copy(qkt[:], tg[:])

                        first = (c == 0 and pb == 0)
                        last = (c == NCH - 1 and pb == PX // 128 - 1)
                        nc.tensor.matmul(gA[:], qkt[:, 0:96], qkt[:, 192:288],
                                         start=first, stop=last)
                        nc.tensor.matmul(gB[:], qkt[:, 96:192], qkt[:, 288:384],
                                         start=first, stop=last)

                # fold chunk partials; pack stats = [gA | gB | ssq]
                for ct in range(3):
                    nc.vector.tensor_reduce(
                        out=stats[0:CT[ct], 192 + ct:193 + ct],
                        in_=sq_part[ct][:], axis=AX.X, op=OP.add)
                nc.scalar.copy(stats[0:96, 0:96], gA[:])
                nc.scalar.copy(stats[0:96, 96:192], gB[:])

            # ================= collective =================
            cc_in = dram.tile([128, 195], F32)
            cc_out = dram.tile([128, 195], F32)
            nc.sync.dma_start(out=cc_in[:], in_=stats[:])
            nc.gpsimd.collective_compute(
                "AllReduce", OP.add, replica_groups=PAIRS,
                ins=[cc_in.opt()], outs=[cc_out.opt()])
            statf = wp.tile([128, 195], F32)
            nc.sync.dma_start(out=statf[:], in_=cc_out[:])

            # ================= epilogue (tiny) =================
            with tc.tile_pool(name="ep", bufs=1) as ep, \
                 tc.tile_pool(name="eps", bufs=1, space="PSUM") as eps:
                # 1/max(sqrt(ssq), eps) per q/k channel, [128, 3] by ct
                nrm = ep.tile([128, 3], F32)
                nc.scalar.activation(out=nrm[:], in_=statf[:, 192:195], func=ACTF.Sqrt)
                nc.vector.tensor_scalar(out=nrm[:], in0=nrm[:], scalar1=1e-12,
                                        scalar2=None, op0=OP.max)
                rn = ep.tile([128, 3], F32)
                nc.vector.reciprocal(rn[:], nrm[:])

                # row scales (q-norms * temperature), partition-packed per gram tile
                rsA = ep.tile([96, 1], F32)
                nc.vector.tensor_tensor(out=rsA[:], in0=rn[0:96, 0:1],
                                        in1=tempcol[0:96, 0:1], op=OP.mult)
                # partition-offset rearrangements go through SBUF->SBUF DMA:
                # DVE writes at non-quadrant-aligned partition bases are illegal
                rsB = ep.tile([96, 1], F32)
                nc.sync.dma_start(out=rsB[0:32, :], in_=rn[96:128, 0:1])
                nc.sync.dma_start(out=rsB[32:96, :], in_=rn[0:64, 1:2])
                nc.vector.tensor_tensor(out=rsB[:], in0=rsB[:],
                                        in1=tempcol[0:96, 1:2], op=OP.mult)

                # column scales (k-norms) -> broadcast [96, 96] via rank-1 matmul
                rkc = ep.tile([96, 2], F32)
                nc.sync.dma_start(out=rkc[0:64, 0:1], in_=rn[64:128, 1:2])
                nc.sync.dma_start(out=rkc[64:96, 0:1], in_=rn[0:32, 2:3])
                nc.sync.dma_start(out=rkc[0:96, 1:2], in_=rn[32:128, 2:3])
                tps = eps.tile([2, 96], F32, tag="t")
                nc.tensor.transpose(tps[:], rkc[:], ident32[0:96, 0:96])
                rkrs = ep.tile([2, 96], F32)
                nc.vector.tensor_copy(rkrs[:], tps[:])
                rkr = [ep.tile([1, 96], F32, name=f"rkr{g}") for g in range(2)]
                nc.vector.tensor_copy(rkr[0][:], rkrs[0:1, :])
                nc.sync.dma_start(out=rkr[1][:], in_=rkrs[1:2, :])
                bcps = eps.tile([96, 96], F32, tag="bc")
                bc = [ep.tile([96, 96], F32, name=f"bc{g}") for g in range(2)]
                for g in range(2):
                    nc.tensor.matmul(bcps[:], ones32[0:1, 0:96], rkr[g][:],
                                     start=True, stop=True)
                    nc.vector.tensor_copy(bc[g][:], bcps[:])

                # logits = gram * rq * rk * temp; diag-extract -> [96, 24] per tile
                attn = []
                for g in range(2):
                    lg = ep.tile([96, 96], F32, name=f"lg{g}")
                    nc.vector.tensor_scalar(out=lg[:], in0=statf[0:96, 96 * g:96 * (g + 1)],
                                            scalar1=(rsA if g == 0 else rsB)[:],
                                            scalar2=None, op0=OP.mult)
                    nc.vector.tensor_tensor(out=lg[:], in0=lg[:], in1=bc[g][:], op=OP.mult)
                    sm = ep.tile([96, HD], F32, name=f"sm{g}")
                    for hl in range(4):
                        nc.sync.dma_start(out=sm[24 * hl:24 * (hl + 1), :],
                                          in_=lg[24 * hl:24 * (hl + 1), 24 * hl:24 * (hl + 1)])
                    mx = ep.tile([96, 1], F32, name=f"mx{g}")
                    nc.vector.tensor_reduce(out=mx[:], in_=sm[:], axis=AX.X, op=OP.max)
                    nc.vector.tensor_scalar(out=sm[:], in0=sm[:], scalar1=mx[:],
                                            scalar2=None, op0=OP.subtract)
                    ex = ep.tile([96, HD], F32, name=f"ex{g}")
                    nc.scalar.activation(out=ex[:], in_=sm[:], func=ACTF.Exp)
                    sme = ep.tile([96, 1], F32, name=f"sme{g}")
                    nc.vector.tensor_reduce(out=sme[:], in_=ex[:], axis=AX.X, op=OP.add)
                    rs = ep.tile([96, 1], F32, name=f"rs{g}")
                    nc.vector.reciprocal(rs[:], sme[:])
                    at = ep.tile([96, HD], BF16, name=f"at{g}")
                    nc.vector.tensor_scalar(out=at[:], in0=ex[:], scalar1=rs[:],
                                            scalar2=None, op0=OP.mult)
                    attn.append(at)

                # blockdiag(attn) as lhsT rows=out-chan(24h+d), cols=v-chan(24h+e)
                abd0 = ep.tile([128, C], BF16)
                abd1 = ep.tile([64, C], BF16)
                nc.vector.memset(abd0[:], 0.0)
                nc.vector.memset(abd1[:], 0.0)
                for h in range(HEADS):
                    g, hl = divmod(h, 4)
                    src = attn[g]
                    r0, cc0 = 24 * h, 24 * h
                    if r0 + 24 <= 128:
                        nc.sync.dma_start(out=abd0[r0:r0 + 24, cc0:cc0 + 24],
                                          in_=src[24 * hl:24 * hl + 24, :])
                    elif r0 >= 128:
                        nc.sync.dma_start(out=abd1[r0 - 128:r0 - 104, cc0:cc0 + 24],
                                          in_=src[24 * hl:24 * hl + 24, :])
                    else:
                        k0 = 128 - r0
                        nc.sync.dma_start(out=abd0[r0:128, cc0:cc0 + 24],
                                          in_=src[24 * hl:24 * hl + k0, :])
                        nc.sync.dma_start(out=abd1[0:24 - k0, cc0:cc0 + 24],
                                          in_=src[24 * hl + k0:24 * hl + 24, :])

                # W2T[c, o] = sum_r abd[r, c] * projt[r, o]
                w2t0 = ep.tile([128, C], BF16)
                w2t1 = ep.tile([64, C], BF16)
                wps = eps.tile([128, C], F32, tag="wps")
                nc.tensor.matmul(wps[:], abd0[:, 0:128], projt0[:], start=True, stop=False)
                nc.tensor.matmul(wps[:], abd1[:, 0:128], projt1[:], start=False, stop=True)
                nc.scalar.copy(w2t0[:], wps[:])
                wps2 = eps.tile([64, C], F32, tag="wps2")
                nc.tensor.matmul(wps2[:], abd0[:, 128:192], projt0[:], start=True, stop=False)
                nc.tensor.matmul(wps2[:], abd1[:, 128:192], projt1[:], start=False, stop=True)
                nc.scalar.copy(w2t1[:], wps2[:])

                # ================= phase 2: out = W2 @ v =================
                with tc.tile_pool(name="p2", bufs=2) as p2, \
                     tc.tile_pool(name="ps2", bufs=2, space="PSUM") as ps2:
                    for c in range(NCH):
                        cs = slice(c * PX, (c + 1) * PX)
                        vb0 = p2.tile([128, PX], BF16, tag="vb0")
                        vb1 = p2.tile([64, PX], BF16, tag="vb1")
                        nc.sync.dma_start(out=vb0[:], in_=v_dram[0:128, cs])
                        nc.sync.dma_start(out=vb1[:], in_=v_dram[128:192, cs])
                        ob0 = p2.tile([128, PX], BF16, tag="ob0")
                        ob1 = p2.tile([64, PX], BF16, tag="ob1")
                        for nb in range(PX // NB):
                            nbs = slice(nb * NB, (nb + 1) * NB)
                            f0 = ps2.tile([128, NB], F32, tag="f0")
                            nc.tensor.matmul(f0[:], w2t0[:, 0:128], vb0[:, nbs], start=True, stop=False)
                            nc.tensor.matmul(f0[:], w2t1[:, 0:128], vb1[:, nbs], start=False, stop=True)
                            nc.scalar.copy(ob0[:, nbs], f0[:])
                            f1 = ps2.tile([64, NB], F32, tag="f1")
                            nc.tensor.matmul(f1[:], w2t0[:, 128:192], vb0[:, nbs], start=True, stop=False)
                            nc.tensor.matmul(f1[:], w2t1[:, 128:192], vb1[:, nbs], start=False, stop=True)
                            nc.scalar.copy(ob1[:, nbs], f1[:])
                        nc.sync.dma_start(out=out_ext[0:128, cs], in_=ob0[:])
                        nc.sync.dma_start(out=out_ext[128:192, cs], in_=ob1[:])
    return nc


_NC_CACHE = None


def _get_nc():
    global _NC_CACHE
    if _NC_CACHE is None:
        _NC_CACHE = build_nc()
    return _NC_CACHE


def _shard_inputs(x, qkv_w, dw_w, proj_w, temperature):
    qkvwt = np.ascontiguousarray(qkv_w.T).astype(ml_dtypes.bfloat16)
    projt = np.ascontiguousarray(proj_w.T).astype(ml_dtypes.bfloat16)
    dw9_ = dw_w.reshape(C3, 9).astype(np.float32)
    dw9 = np.ascontiguousarray(np.concatenate([dw9_, -dw9_], axis=1))
    # per-PE-tap diagonal weight blocks: dwdiag[s, i, CTO[ct]+i] = w(tap_s, ch)
    dwdiag = np.zeros((len(PE_TAPS), 128, C3), np.float32)
    for s, (dy, dx) in enumerate(PE_TAPS):
        wv = dw9_[:, 3 * dy + dx]
        for ct in range(5):
            idx = np.arange(CT[ct])
            dwdiag[s, idx, CTO[ct] + idx] = wv[CTO[ct] + idx]
    dwdiag = dwdiag.reshape(len(PE_TAPS) * 128, C3).astype(ml_dtypes.bfloat16)
    temp = np.asarray(temperature).reshape(HEADS)
    tempcol = np.zeros((128, 2), np.float32)
    for h in range(HEADS):
        g, hl = divmod(h, 4)
        tempcol[24 * hl:24 * (hl + 1), g] = temp[h]

    in_maps = []
    for i in range(8):
        b, hf = divmod(i, 2)
        xin = np.zeros((C, HALF + 2, W), np.float32)
        r0 = hf * HALF - 1
        lo, hi = max(r0, 0), min(r0 + HALF + 2, H)
        xin[:, lo - r0:hi - r0, :] = x[b, :, lo:hi, :]
        in_maps.append({
            "xin": xin.reshape(C, (HALF + 2) * W).astype(ml_dtypes.bfloat16),
            "qkvwt": qkvwt, "projt": projt, "dw9": dw9, "tempcol": tempcol,
            "dwdiag": dwdiag,
        })
    return in_maps


def kernel(x, qkv_w, dw_w, proj_w, temperature):
    nc = _get_nc()
    in_maps = _shard_inputs(x, qkv_w, dw_w, proj_w, temperature)
    res = run_bass_kernel_spmd(nc, in_maps, core_ids=list(range(8)))
    out = np.empty((B, C, H, W), np.float32)
    for i in range(8):
        b, hf = divmod(i, 2)
        o = res.results[i]["out"].astype(np.float32).reshape(C, HALF, W)
        out[b, :, hf * HALF:(hf + 1) * HALF, :] = o
    return out
